# revision 39
# baseline (speedup 1.0000x reference)
"""Trainium2 Bass kernel for nn_DensePoseV1ConvXGNSparseHead.

8 layers of submanifold 3x3 conv (gather-GEMM over 9 taps) + GroupNorm(32)
+ ReLU on N=131072 sparse sites, 256->512 then 512->512 channels.

Strategy
--------
The 9-tap rulebook is a 3x3 stencil on a ~60%-occupied grid.  On the host we
reconstruct a planar embedding of the points from nbr_idx (min-label
propagation over the neighbor graph), pack the connected components into a
padded dense canvas, and run the conv as *dense* channel-major bf16 matmuls
with shifted access patterns: zero gather, zero transposes, contiguous DMA.
Inactive/pad cells are kept at exactly 0 by folding an activity mask into
the GroupNorm affine application, so submanifold semantics are preserved.

Sharding: canvas rows are split across the 8 cores with an 8-row halo on
each side - the full receptive field of 8 stacked 3x3 convs - so every core
computes its slice for all 8 layers with ZERO inter-core communication.
Conv weights / GN params are replicated (sharding_hint's halo all-gather is
avoided entirely by recomputing the halo locally).

Per layer, per col-block, per 128-channel output chunk:
  - conv: 4ci x 9tap x subtile accumulating bf16 matmuls into PSUM
  - GroupNorm stats as one PE matmul chain over stacked [y | y^2] moving
    data with 0/1 group masks (partition reduce), landing on partitions
    32:64 so the mask (0:32) + w32 (32:64) stack feeds a single
    64-contraction B matmul
  - rsqrt via DVE reciprocal + ACT sqrt; normalize folded into two
    PE-broadcast matmuls (A = gamma*inv*mask, B = beta*mask - gamma*mu*inv*mask)
  - apply y*A+B on DVE, ReLU on ACT (cast to bf16 for the next layer).
fp32 is kept through GroupNorm stats/apply; conv inputs are bf16 (the PE
runs bf16 at the same rows/cycle but with half the LDWEIGHTS cost and much
less power -> less hardware util-throttling than fp32r).
"""

import numpy as np

import concourse.bass as bass
import concourse.tile as tile
from concourse import bacc, mybir
from concourse.bass_utils import run_bass_kernel_spmd

DT = mybir.dt
F32R = DT.float32r
BF16 = DT.bfloat16

N_TAPS = 9
OFFS = [(dy, dx) for dy in (-1, 0, 1) for dx in (-1, 0, 1)]
OFFS_ARR = np.array(OFFS, np.int64)
HALO_ROWS = 8
N_CORES = 8
BLOCK = 1536
SUB = 512  # psum subtile (fp32 bank)
NSUB = BLOCK // SUB
HA = 1024  # first conv half (psum tag ca, 2 banks); second half is SUB
HID = 512
GSIZE = 16
EPS = 1e-5
CPAD = 128  # zero columns left/right of the compute region (conv reads +-67)
WIN = BLOCK + 2 * 67  # x window per block


# ----------------------------------------------------------------- host side

def _embed_points(nbr):
    n = nbr.shape[0]
    assert nbr.shape[1] == N_TAPS
    assert (nbr[:, 4] == np.arange(n)).all(), "tap 4 must be self"
    comp = np.arange(n, dtype=np.int64)
    py = np.zeros(n, np.int64)
    px = np.zeros(n, np.int64)
    edges = []
    for k in range(N_TAPS):
        if k == 4:
            continue
        t = nbr[:, k]
        src = np.flatnonzero(t >= 0)
        edges.append((src, t[src].astype(np.int64), int(OFFS_ARR[k, 0]),
                      int(OFFS_ARR[k, 1])))
    for _ in range(100_000):
        changed = False
        for src, dst, dy, dx in edges:
            bad = comp[src] < comp[dst]
            if bad.any():
                s, d = src[bad], dst[bad]
                order = np.argsort(comp[s], kind="stable")
                s, d = s[order], d[order]
                uniq, first = np.unique(d, return_index=True)
                s, d = s[first], uniq
                comp[d] = comp[s]
                py[d] = py[s] + dy
                px[d] = px[s] + dx
                changed = True
        if not changed:
            break
    else:
        raise RuntimeError("label propagation did not converge")
    for k in range(N_TAPS):
        t = nbr[:, k]
        src = np.flatnonzero(t >= 0)
        dst = t[src]
        ok = ((comp[src] == comp[dst])
              & (py[dst] == py[src] + OFFS_ARR[k, 0])
              & (px[dst] == px[src] + OFFS_ARR[k, 1]))
        if not ok.all():
            raise RuntimeError(f"rulebook inconsistent at tap {k}")
    return comp, py, px


def _build_canvas_map(nbr):
    n = nbr.shape[0]
    comp, py, px = _embed_points(nbr)
    uniq, inv = np.unique(comp, return_inverse=True)
    ncmp = uniq.size
    big = 1 << 60
    miny = np.full(ncmp, big); minx = np.full(ncmp, big)
    maxy = np.full(ncmp, -big); maxx = np.full(ncmp, -big)
    np.minimum.at(miny, inv, py); np.minimum.at(minx, inv, px)
    np.maximum.at(maxy, inv, py); np.maximum.at(maxx, inv, px)
    h = maxy - miny + 1
    w = maxx - minx + 1
    # one guard column per row: col 0 of every row stays empty, and
    # dx=+1 reads off col stride-1 wrap onto the next row's guard col.
    stride = int(w.max()) + 1
    shelf_w = stride - 1

    # Pack components: big ones stacked vertically (full rows); small ones
    # shelf-packed side by side to avoid burning a full canvas row each.
    npts = np.bincount(inv)
    isbig = npts > 1000
    row_off = np.zeros(ncmp, np.int64)
    col_off = np.ones(ncmp, np.int64)
    acc = 0
    for c in np.flatnonzero(isbig):
        row_off[c] = acc
        acc += int(h[c]) + 1
    order = sorted(np.flatnonzero(~isbig), key=lambda c: -int(h[c]))
    shelf_row, shelf_h, xcur = acc, 0, 0
    for c in order:
        if xcur + int(w[c]) > shelf_w:
            shelf_row += shelf_h + 1
            shelf_h, xcur = 0, 0
        if shelf_h == 0:
            shelf_h = int(h[c])
        row_off[c] = shelf_row
        col_off[c] = 1 + xcur
        xcur += int(w[c]) + 1
    if xcur > 0:
        shelf_row += shelf_h + 1
    total_rows = int(shelf_row)
    r8 = -(-total_rows // N_CORES)
    rg = N_CORES * r8 + 2 * HALO_ROWS
    grow = HALO_ROWS + row_off[inv] + (py - miny[inv])
    gcol = col_off[inv] + (px - minx[inv])
    pos = grow * stride + gcol
    occupied = np.zeros(rg * stride, bool)
    if pos.max() >= occupied.size or np.unique(pos).size != n:
        raise RuntimeError("canvas build failed")
    occupied[pos] = True
    for k in range(N_TAPS):
        if k == 4:
            continue
        dpos = int(OFFS_ARR[k, 0]) * stride + int(OFFS_ARR[k, 1])
        if occupied[pos[nbr[:, k] < 0] + dpos].any():
            raise RuntimeError(f"tap {k}: active cell where rulebook says -1")
    m_raw = (r8 + 2 * HALO_ROWS) * stride
    m_pad = -(-m_raw // SUB) * SUB  # 512-granular; last block may be short
    nfull = m_pad // BLOCK
    tail_sub = (m_pad - nfull * BLOCK) // SUB
    return pos, dict(stride=stride, r8=r8, rg=rg, m_raw=m_raw, m_pad=m_pad,
                     nfull=nfull, tail_sub=tail_sub)


# --------------------------------------------------------------- bass program

def _build_program(m_pad, nfull, tail_sub, layers, stride):
    # extra tail slack so window loads may harmlessly overread
    padw = CPAD + m_pad + CPAD + BLOCK + 128
    n_eff = nfull + (1 if tail_sub else 0)
    nc = bacc.Bacc("TRN2", target_bir_lowering=False, debug=False)

    x0_d = nc.dram_tensor("x0", (2, 128, padw), BF16, kind="ExternalInput")
    w0_d = nc.dram_tensor("w0p", (128, N_TAPS * 2 * HID), BF16,
                          kind="ExternalInput")
    wr_d = nc.dram_tensor("wrp", (max(layers - 1, 1), 128, N_TAPS * 4 * HID),
                          BF16, kind="ExternalInput")
    acg_d = nc.dram_tensor("acg", (layers, 32, 2048), F32R, kind="ExternalInput")
    bcgm_d = nc.dram_tensor("bcgm", (layers, 64, 2048), F32R,
                            kind="ExternalInput")
    smask_d = nc.dram_tensor("smask", (128, 256), F32R, kind="ExternalInput")
    msk64_d = nc.dram_tensor("msk64", (64, (nfull + 2) * SUB), F32R,
                             kind="ExternalInput")
    out_d = nc.dram_tensor("out", (4, 128, m_pad), DT.float32,
                           kind="ExternalOutput")
    xa_d = nc.dram_tensor("xa", (4, 128, padw), BF16, kind="Internal")
    xb_d = nc.dram_tensor("xb", (4, 128, padw), BF16, kind="Internal")

    deltas = [dy * stride + dx for dy, dx in OFFS]

    with tile.TileContext(nc) as tc:
        with (
            tc.tile_pool(name="consts", bufs=1) as constp,
            tc.tile_pool(name="wp", bufs=2) as wpool,
            tc.tile_pool(name="lyc", bufs=1) as lycp,
            tc.tile_pool(name="yb", bufs=3) as ypool,
            tc.tile_pool(name="ybf", bufs=2) as ybfpool,
            tc.tile_pool(name="tt", bufs=1) as ttpool,
            tc.tile_pool(name="tt2", bufs=2) as tt2pool,
            tc.tile_pool(name="psC", bufs=1, space=bass.MemorySpace.PSUM) as psCp,
            tc.tile_pool(name="psS", bufs=1, space=bass.MemorySpace.PSUM) as psSp,
            tc.tile_pool(name="psA", bufs=1, space=bass.MemorySpace.PSUM) as psAp,
            tc.tile_pool(name="psB", bufs=2, space=bass.MemorySpace.PSUM) as psBp,
        ):
            smask = constp.tile([128, 256], F32R)
            nc.sync.dma_start(smask[:], smask_d.ap())
            xw0 = constp.tile([128, 4 * WIN], BF16, tag="xw0")
            xw1 = constp.tile([128, 4 * WIN], BF16, tag="xw1")
            xwt = [xw0, xw1]
            # mask replicated in rows 0:32 and 32:64 per (parity, co);
            # ep_stats overwrites rows 0:32 with w32 so [w32; msk] pairs
            # with the [gm16; bc32] stack in one 64-contraction B matmul
            mwt = [[], []]
            for p in range(2):
                for co in range(4):
                    mw_pc = constp.tile([64, SUB], F32R, tag=f"mw{p}{co}",
                                        name=f"mw{p}{co}")
                    mwt[p].append(mw_pc)

            # zero the conv pads of the internal ping-pong buffers once
            zpad = constp.tile([128, CPAD], BF16)
            nc.gpsimd.memset(zpad[:], 0.0)
            for buf in (xa_d, xb_d):
                for ci in range(4):
                    nc.sync.dma_start(buf.ap()[ci, :, 0:CPAD], zpad[:])
                    for z0 in range(CPAD + m_pad, padw, CPAD):
                        zw = min(CPAD, padw - z0)
                        nc.sync.dma_start(buf.ap()[ci, :, z0:z0 + zw],
                                          zpad[:, 0:zw])

            state = {"cur": 0}

            def load_xw(p, src_aps, nci_, bexpr):
                for ci in range(nci_):
                    nc.sync.dma_start(
                        xwt[p][:, ci * WIN:(ci + 1) * WIN],
                        src_aps[ci][:, bass.ds(bexpr * BLOCK + (CPAD - 67),
                                               WIN)])

            def load_masks(p, bexpr):
                for co in range(4):
                    nc.sync.dma_start(
                        mwt[p][co][:],
                        msk64_d.ap()[:, bass.ds(bexpr * SUB, SUB)])

            def load_weights(li):
                nci_ = 2 if li == 0 else 4
                w_sb = wpool.tile([128, N_TAPS * 4 * HID], BF16, tag="w")
                wsrc = (w0_d.ap() if li == 0
                        else wr_d.ap()[li - 1, :, 0:N_TAPS * 4 * HID])
                wq = N_TAPS * nci_ * 128  # cols per co chunk
                for co in range(4):
                    nc.gpsimd.dma_start(w_sb[:, co * wq:(co + 1) * wq],
                                        wsrc[:, co * wq:(co + 1) * wq])
                return w_sb

            # GroupNorm epilogue queues, software-pipelined across block AND
            # layer boundaries so the PE never drains at either.
            ep_q = {"stat": [], "ab": []}

            def ep_push(stat_fn):
                ep_q["stat"].append(stat_fn)
                if len(ep_q["stat"]) > 1:
                    ep_q["ab"].append(ep_q["stat"].pop(0)())
                if len(ep_q["ab"]) > 1:
                    ep_q["ab"].pop(0)()

            def ep_drain():
                while ep_q["stat"]:
                    ep_q["ab"].append(ep_q["stat"].pop(0)())
                    if len(ep_q["ab"]) > 1:
                        ep_q["ab"].pop(0)()
                while ep_q["ab"]:
                    ep_q["ab"].pop(0)()

            def run_layer(li, nci, src_aps, dst_aps, final, w_sb, w_next):
                acg_sb = lycp.tile([32, 2048], F32R, tag="acg")
                nc.gpsimd.dma_start(acg_sb[:], acg_d.ap()[li])
                bcgm_sb = lycp.tile([64, 2048], F32R, tag="bcgm")
                nc.gpsimd.dma_start(bcgm_sb[:], bcgm_d.ap()[li])

                def conv_half(co, j0, j1, ps, cur):
                    nmm = nci * N_TAPS
                    mi = 0
                    for ci in range(nci):
                        for k in range(N_TAPS):
                            woff = (co * nci * N_TAPS + k * nci + ci) * 128
                            lhsT = w_sb[:, woff:woff + 128]
                            base = ci * WIN + 67 + deltas[k]
                            for j in range(j0, j1):
                                nc.tensor.matmul(
                                    ps[:, (j - j0) * SUB:(j - j0 + 1) * SUB],
                                    lhsT,
                                    xwt[cur][:, base + j * SUB:
                                              base + j * SUB + SUB],
                                    start=(mi == 0), stop=(mi == nmm - 1))
                            mi += 1

                def ep_stats(co, y2t, mw_t, nsub):
                    # psX2[:, 0:512] = sum y per group; [:, 512:1024] = E[y^2]
                    # (the h=1 smask slices carry the 1/16 scaling)
                    psX2 = psSp.tile([32, 2 * SUB], DT.float32, tag="sxx")
                    for h in range(2):
                        for j in range(nsub):
                            nc.tensor.matmul(psX2[:, h * SUB:(h + 1) * SUB],
                                             smask[:, h * 128 + j * 32:
                                                   h * 128 + (j + 1) * 32],
                                             y2t[:, h, j * SUB:(j + 1) * SUB],
                                             start=(j == 0),
                                             stop=(j == nsub - 1))
                    sxs = ttpool.tile([32, SUB], F32R, tag="sxs")
                    nc.vector.tensor_copy(sxs[:], psX2[:, 0:SUB])
                    a = ttpool.tile([32, SUB], DT.float32, tag="a")
                    nc.vector.scalar_tensor_tensor(
                        a[:], sxs[:], -1.0 / (GSIZE * GSIZE), sxs[:],
                        mybir.AluOpType.mult, mybir.AluOpType.mult)  # -mu^2
                    uu = ttpool.tile([32, SUB], DT.float32, tag="uu")
                    nc.vector.scalar_tensor_tensor(
                        uu[:], psX2[:, SUB:2 * SUB], EPS, a[:],
                        mybir.AluOpType.add, mybir.AluOpType.add)  # var+eps
                    r = ttpool.tile([32, SUB], DT.float32, tag="r")
                    nc.vector.reciprocal_approx_fast(r[:], uu[:])
                    inv = ttpool.tile([32, SUB], DT.float32, tag="a")
                    nc.scalar.activation(inv[:], r[:],
                                         mybir.ActivationFunctionType.Sqrt)
                    invm = tt2pool.tile([32, SUB], F32R, tag="invm")
                    nc.vector.tensor_tensor(invm[:], inv[:], mw_t[0:32, :],
                                            mybir.AluOpType.mult)
                    # w32 = (sum y)*inv*msk overwrites the rows-0:32 mask copy
                    nc.vector.tensor_tensor(mw_t[0:32, :], sxs[:], invm[:],
                                            mybir.AluOpType.mult)
                    return invm, mw_t

                def ep_ab(co, y2t, invm, mw_t, bexpr, nsub):
                    ybf = (None if final
                           else ybfpool.tile([128, BLOCK], BF16, tag="ybf"))
                    for j in range(nsub):
                        cj = co * 512 + j * 128
                        psA = psAp.tile([128, SUB], DT.float32, tag="A")
                        nc.tensor.matmul(psA[:], acg_sb[:, cj:cj + 128],
                                         invm[:], start=True, stop=True)
                        psB = psBp.tile([128, SUB], DT.float32, tag="B")
                        nc.tensor.matmul(psB[:], bcgm_sb[:, cj:cj + 128],
                                         mw_t[:], start=True, stop=True)
                        t1 = tt2pool.tile([128, SUB], DT.float32, tag="t1")
                        nc.vector.tensor_tensor(
                            t1[:], psA[:], y2t[:, 0, j * SUB:(j + 1) * SUB],
                            mybir.AluOpType.mult)
                        t2 = tt2pool.tile([128, SUB], DT.float32, tag="t2")
                        nc.vector.tensor_tensor(t2[:], psB[:], t1[:],
                                                mybir.AluOpType.add)
                        nc.scalar.activation(
                            (y2t[:, 0, j * SUB:(j + 1) * SUB] if final
                             else ybf[:, j * SUB:(j + 1) * SUB]),
                            t2[:], mybir.ActivationFunctionType.Relu)

                    dst = dst_aps[co][:, bass.ds(bexpr * BLOCK + (0 if final
                                                                  else CPAD),
                                                 nsub * SUB)]
                    # trigger output writes from the ACT engine's DGE queue so
                    # the sync queue carries only loads (keeps the window
                    # prefetch from queuing behind this block's writes)
                    nc.scalar.dma_start(
                        dst,
                        ybf[:, 0:nsub * SUB] if not final
                        else y2t[:, 0, 0:nsub * SUB].bitcast(DT.float32))

                def run_block(bexpr, nsub, pre):
                    cur = state["cur"]
                    if pre is not None:
                        if pre[0] == "same":
                            load_xw(1 - cur, src_aps, nci, pre[1])
                        else:  # next layer's block 0 (reads this layer's dst)
                            load_xw(1 - cur, pre[1], 4, 0)
                    for co in range(4):
                        y2t = ypool.tile([128, 2, BLOCK], F32R, tag="y")
                        haj = min(2, nsub)
                        psa = psCp.tile([128, HA], DT.float32, tag="ca")
                        conv_half(co, 0, haj, psa, cur)
                        nc.vector.tensor_copy(y2t[:, 0, 0:haj * SUB],
                                              psa[:, 0:haj * SUB])
                        nc.scalar.square(y2t[:, 1, 0:haj * SUB],
                                         y2t[:, 0, 0:haj * SUB])
                        if nsub == 3:
                            psb = psCp.tile([128, SUB], DT.float32, tag="cb")
                            conv_half(co, 2, 3, psb, cur)
                            nc.vector.tensor_copy(y2t[:, 0, HA:BLOCK], psb[:])
                            nc.scalar.square(y2t[:, 1, HA:BLOCK],
                                             y2t[:, 0, HA:BLOCK])

                        def mk_stat(co=co, y2t=y2t, mw_t=mwt[cur][co],
                                    bexpr=bexpr, nsub=nsub):
                            def do_stat():
                                st = ep_stats(co, y2t, mw_t, nsub)

                                def do_ab():
                                    ep_ab(co, y2t, *st, bexpr, nsub)
                                return do_ab
                            return do_stat

                        ep_push(mk_stat())
                    # mask prefetch emitted after the carried ep work of the
                    # previous block (same tile parity) to keep emission
                    # order consistent with the semantic order
                    if pre is not None:
                        load_masks(1 - cur, 0 if pre[0] == "next" else pre[1])
                    state["cur"] = 1 - cur

                # For_i body of 6 blocks (6 is a multiple of every pool's
                # bufs count, so carried tile handles stay phase-aligned);
                # trailing blocks in python carry the cross-layer prefetch.
                n_loop = ((n_eff - 1) // 6) * 6
                if n_loop:
                    with tc.For_i(0, n_loop, 6,
                                  hint_engines=(mybir.EngineType.PE,)) as i:
                        for u in range(6):
                            run_block(i + u, 3, ("same", i + u + 1))
                        # the body must enter/exit with an empty ep queue so
                        # tile handles never cross a loop-iteration boundary
                        ep_drain()
                for b in range(n_loop, n_eff):
                    nsub = 3 if b < nfull else tail_sub
                    if b < n_eff - 1:
                        pre = ("same", b + 1)
                    elif not final:
                        pre = ("next", dst_aps)
                    else:
                        pre = None
                    run_block(b, nsub, pre)
                ep_drain()

            bufs = {"x0": x0_d, "xa": xa_d, "xb": xb_d}
            seq = ["x0"] + ["xa", "xb"] * 4
            w_cur = load_weights(0)
            load_xw(0, [x0_d.ap()[ci] for ci in range(2)], 2, 0)
            load_masks(0, 0)
            for li in range(layers):
                src, dst = seq[li], seq[li + 1]
                nci = 2 if li == 0 else 4
                src_aps = [bufs[src].ap()[ci] for ci in range(nci)]
                final = li == layers - 1
                dst_aps = ([out_d.ap()[co] for co in range(4)] if final
                           else [bufs[dst].ap()[co] for co in range(4)])
                w_next = None if final else load_weights(li + 1)
                run_layer(li, nci, src_aps, dst_aps, final, w_cur, w_next)
                w_cur = w_next
            ep_drain()

    nc.compile()
    return nc


# ------------------------------------------------------------- host packing

def _pack_host(inputs, pos, meta, layers=8):
    bf16 = mybir.dt.np(BF16)
    feats = np.ascontiguousarray(np.asarray(inputs["features"], np.float32))
    w0 = np.asarray(inputs["w0"], np.float32)
    w_rest = np.asarray(inputs["w_rest"], np.float32)
    gamma = np.asarray(inputs["gamma"], np.float32)
    beta = np.asarray(inputs["beta"], np.float32)
    n, cin = feats.shape
    stride, r8, m_pad = meta["stride"], meta["r8"], meta["m_pad"]
    nfull = meta["nfull"]
    rgst = meta["rg"] * stride
    padw = CPAD + m_pad + CPAD + BLOCK + 128

    x_g = np.zeros((cin, rgst), np.float32)
    x_g[:, pos] = feats.T
    mask_g = np.zeros(rgst, np.float32)
    mask_g[pos] = 1.0

    # weights: wpk[p, co, k, ci, :] = w[k, ci*128+p, co*128:(co+1)*128]
    w0p = np.ascontiguousarray(
        w0.reshape(N_TAPS, 2, 128, 4, 128).transpose(2, 3, 0, 1, 4)
    ).reshape(128, N_TAPS * 2 * HID).astype(bf16)
    nl = max(layers - 1, 1)
    wrp = np.ascontiguousarray(
        w_rest[:layers - 1].reshape(layers - 1, N_TAPS, 4, 128, 4, 128)
        .transpose(0, 3, 4, 1, 2, 5)
    ).reshape(layers - 1, 128, N_TAPS * 4 * HID).astype(bf16)
    if wrp.shape[0] < nl:
        wrp = np.zeros((nl, 128, N_TAPS * 4 * HID), bf16)

    ch = np.arange(128)
    acg = np.zeros((layers, 32, 4, 4, 128), np.float32)
    gm16 = np.zeros((layers, 32, 4, 4, 128), np.float32)
    bc32 = np.zeros((layers, 32, 4, 4, 128), np.float32)
    for li in range(layers):
        for co in range(4):
            g_ = gamma[li, co * 128:(co + 1) * 128]
            b_ = beta[li, co * 128:(co + 1) * 128]
            for j in range(4):
                rows = 8 * j + ch // GSIZE
                acg[li, rows, co, j, ch] = g_
                gm16[li, rows, co, j, ch] = -g_ / GSIZE
                bc32[li, 8 * j, co, j, :] = b_
    acg = acg.reshape(layers, 32, 2048)
    # B matmul stack: rows 0:32 pair with w32, rows 32:64 with the mask
    bcgm = np.concatenate([gm16.reshape(layers, 32, 2048),
                           bc32.reshape(layers, 32, 2048)], axis=1)

    # stats mask: out partition 8j + g sums group g of subtile j.
    # cols 0:128 (h=0, sum y) carry 1.0; cols 128:256 (h=1) carry 1/16
    # so the second stats chain lands as E[y^2] directly.
    smask = np.zeros((2, 128, 4, 32), np.float32)
    for j in range(4):
        smask[0, ch, j, 8 * j + ch // GSIZE] = 1.0
        smask[1, ch, j, 8 * j + ch // GSIZE] = 1.0 / GSIZE
    smask = smask.transpose(1, 0, 2, 3).reshape(128, 256)

    in_maps = []
    mpb = (nfull + 1) * BLOCK
    for s in range(N_CORES):
        c0 = s * r8 * stride
        x0 = np.zeros((2, 128, padw), bf16)
        seg = x_g[:, c0:min(c0 + m_pad, rgst)]
        x0[:, :, CPAD:CPAD + seg.shape[1]] = seg.reshape(2, 128, -1).astype(bf16)
        mc = np.zeros(mpb, np.float32)
        mseg = mask_g[c0:min(c0 + mpb, rgst)]
        mc[:mseg.shape[0]] = mseg
        # msk64[8j+g, b*512+c] = msk64[32+8j+g, ...] = mask[b*1536 + j*512 + c]
        # (rows 0:32 get overwritten by w32 on-chip; rows 32:64 stay mask)
        m4 = mc.reshape(-1, NSUB, SUB)  # [nb, j, c]
        msk64 = np.zeros((64, (nfull + 2) * SUB), np.float32)
        for j in range(NSUB):
            for g in range(8):
                msk64[8 * j + g, :mpb // NSUB] = m4[:, j, :].reshape(-1)
                msk64[32 + 8 * j + g, :mpb // NSUB] = m4[:, j, :].reshape(-1)
        in_maps.append({
            "x0": x0, "w0p": w0p, "wrp": wrp, "acg": acg, "bcgm": bcgm,
            "smask": smask, "msk64": msk64,
        })
    return in_maps


TRACE = False
LAST_RESULT = {}


def kernel(**inputs) -> np.ndarray:
    nbr = np.asarray(inputs["nbr_idx"])
    n = nbr.shape[0]
    pos, meta = _build_canvas_map(nbr)
    in_maps = _pack_host(inputs, pos, meta)
    nc = _build_program(meta["m_pad"], meta["nfull"], meta["tail_sub"], 8,
                        meta["stride"])
    res = run_bass_kernel_spmd(nc, in_maps, list(range(N_CORES)), trace=TRACE)
    LAST_RESULT["exec_time_ns"] = res.exec_time_ns
    LAST_RESULT["profile_json"] = res.profile_json

    stride, r8 = meta["stride"], meta["r8"]
    row = pos // stride
    own = np.clip((row - HALO_ROWS) // r8, 0, N_CORES - 1)
    result = np.zeros((n, HID), np.float32)
    for s in range(N_CORES):
        sel = own == s
        local = pos[sel] - s * r8 * stride
        o = res.results[s]["out"]  # [4, 128, m_pad]
        result[sel] = o[:, :, local].reshape(HID, -1).T
    return result


if __name__ == "__main__":
    import reference

    inputs = reference.setup_inputs()
    out = kernel(**{k: np.asarray(v) for k, v in inputs.items()})
    exp = np.asarray(reference.reference(**inputs))
    err = np.linalg.norm(out - exp) / np.linalg.norm(exp)
    print(f"l2 rel err: {err:.3e}")


# revision 41
# speedup vs baseline: 1.1193x; 1.1193x over previous
"""Trainium2 Bass kernel for nn_DensePoseV1ConvXGNSparseHead.

8 layers of submanifold 3x3 conv (gather-GEMM over 9 taps) + GroupNorm(32)
+ ReLU on N=131072 sparse sites, 256->512 then 512->512 channels.

Strategy
--------
The 9-tap rulebook is a 3x3 stencil on a ~60%-occupied grid.  On the host we
reconstruct a planar embedding of the points from nbr_idx (min-label
propagation over the neighbor graph), pack the connected components into a
padded dense canvas, and run the conv as *dense* channel-major bf16 matmuls
with shifted access patterns: zero gather, zero transposes, contiguous DMA.
Inactive/pad cells are kept at exactly 0 by folding an activity mask into
the GroupNorm affine application, so submanifold semantics are preserved.

Sharding: canvas rows are split across the 8 cores with an 8-row halo on
each side - the full receptive field of 8 stacked 3x3 convs - so every core
computes its slice for all 8 layers with ZERO inter-core communication.
Conv weights / GN params are replicated (sharding_hint's halo all-gather is
avoided entirely by recomputing the halo locally).

Per layer, per col-block, per 128-channel output chunk:
  - conv: 4ci x 9tap x subtile accumulating bf16 matmuls into PSUM
  - GroupNorm stats as one PE matmul chain over stacked [y | y^2] moving
    data with 0/1 group masks (partition reduce), landing on partitions
    32:64 so the mask (0:32) + w32 (32:64) stack feeds a single
    64-contraction B matmul
  - rsqrt via DVE reciprocal + ACT sqrt; normalize folded into two
    PE-broadcast matmuls (A = gamma*inv*mask, B = beta*mask - gamma*mu*inv*mask)
  - apply y*A+B on DVE, ReLU on ACT (cast to bf16 for the next layer).
fp32 is kept through GroupNorm stats/apply; conv inputs are bf16 (the PE
runs bf16 at the same rows/cycle but with half the LDWEIGHTS cost and much
less power -> less hardware util-throttling than fp32r).
"""

import numpy as np

import concourse.bass as bass
import concourse.tile as tile
from concourse import bacc, mybir
from concourse.bass_utils import run_bass_kernel_spmd

DT = mybir.dt
F32R = DT.float32r
BF16 = DT.bfloat16

N_TAPS = 9
OFFS = [(dy, dx) for dy in (-1, 0, 1) for dx in (-1, 0, 1)]
OFFS_ARR = np.array(OFFS, np.int64)
HALO_ROWS = 8
N_CORES = 8
BLOCK = 1536
SUB = 512  # psum subtile (fp32 bank)
NSUB = BLOCK // SUB
HA = 1024  # first conv half (psum tag ca, 2 banks); second half is SUB
HID = 512
GSIZE = 16
EPS = 1e-5
CPAD = 128  # zero columns left/right of the compute region (conv reads +-67)
WIN = BLOCK + 2 * 67  # x window per block


# ----------------------------------------------------------------- host side

def _embed_points(nbr):
    n = nbr.shape[0]
    assert nbr.shape[1] == N_TAPS
    assert (nbr[:, 4] == np.arange(n)).all(), "tap 4 must be self"
    comp = np.arange(n, dtype=np.int64)
    py = np.zeros(n, np.int64)
    px = np.zeros(n, np.int64)
    edges = []
    for k in range(N_TAPS):
        if k == 4:
            continue
        t = nbr[:, k]
        src = np.flatnonzero(t >= 0)
        edges.append((src, t[src].astype(np.int64), int(OFFS_ARR[k, 0]),
                      int(OFFS_ARR[k, 1])))
    for _ in range(100_000):
        changed = False
        for src, dst, dy, dx in edges:
            bad = comp[src] < comp[dst]
            if bad.any():
                s, d = src[bad], dst[bad]
                order = np.argsort(comp[s], kind="stable")
                s, d = s[order], d[order]
                uniq, first = np.unique(d, return_index=True)
                s, d = s[first], uniq
                comp[d] = comp[s]
                py[d] = py[s] + dy
                px[d] = px[s] + dx
                changed = True
        if not changed:
            break
    else:
        raise RuntimeError("label propagation did not converge")
    for k in range(N_TAPS):
        t = nbr[:, k]
        src = np.flatnonzero(t >= 0)
        dst = t[src]
        ok = ((comp[src] == comp[dst])
              & (py[dst] == py[src] + OFFS_ARR[k, 0])
              & (px[dst] == px[src] + OFFS_ARR[k, 1]))
        if not ok.all():
            raise RuntimeError(f"rulebook inconsistent at tap {k}")
    return comp, py, px


def _build_canvas_map(nbr):
    n = nbr.shape[0]
    comp, py, px = _embed_points(nbr)
    uniq, inv = np.unique(comp, return_inverse=True)
    ncmp = uniq.size
    big = 1 << 60
    miny = np.full(ncmp, big); minx = np.full(ncmp, big)
    maxy = np.full(ncmp, -big); maxx = np.full(ncmp, -big)
    np.minimum.at(miny, inv, py); np.minimum.at(minx, inv, px)
    np.maximum.at(maxy, inv, py); np.maximum.at(maxx, inv, px)
    h = maxy - miny + 1
    w = maxx - minx + 1
    # one guard column per row: col 0 of every row stays empty, and
    # dx=+1 reads off col stride-1 wrap onto the next row's guard col.
    stride = int(w.max()) + 1
    shelf_w = stride - 1

    # Pack components: big ones stacked vertically (full rows); small ones
    # shelf-packed side by side to avoid burning a full canvas row each.
    npts = np.bincount(inv)
    isbig = npts > 1000
    row_off = np.zeros(ncmp, np.int64)
    col_off = np.ones(ncmp, np.int64)
    acc = 0
    for c in np.flatnonzero(isbig):
        row_off[c] = acc
        acc += int(h[c]) + 1
    order = sorted(np.flatnonzero(~isbig), key=lambda c: -int(h[c]))
    shelf_row, shelf_h, xcur = acc, 0, 0
    for c in order:
        if xcur + int(w[c]) > shelf_w:
            shelf_row += shelf_h + 1
            shelf_h, xcur = 0, 0
        if shelf_h == 0:
            shelf_h = int(h[c])
        row_off[c] = shelf_row
        col_off[c] = 1 + xcur
        xcur += int(w[c]) + 1
    if xcur > 0:
        shelf_row += shelf_h + 1
    total_rows = int(shelf_row)
    r8 = -(-total_rows // N_CORES)
    rg = N_CORES * r8 + 2 * HALO_ROWS
    grow = HALO_ROWS + row_off[inv] + (py - miny[inv])
    gcol = col_off[inv] + (px - minx[inv])
    pos = grow * stride + gcol
    occupied = np.zeros(rg * stride, bool)
    if pos.max() >= occupied.size or np.unique(pos).size != n:
        raise RuntimeError("canvas build failed")
    occupied[pos] = True
    for k in range(N_TAPS):
        if k == 4:
            continue
        dpos = int(OFFS_ARR[k, 0]) * stride + int(OFFS_ARR[k, 1])
        if occupied[pos[nbr[:, k] < 0] + dpos].any():
            raise RuntimeError(f"tap {k}: active cell where rulebook says -1")
    m_raw = (r8 + 2 * HALO_ROWS) * stride
    m_pad = -(-m_raw // SUB) * SUB  # 512-granular; last block may be short
    nfull = m_pad // BLOCK
    tail_sub = (m_pad - nfull * BLOCK) // SUB
    return pos, dict(stride=stride, r8=r8, rg=rg, m_raw=m_raw, m_pad=m_pad,
                     nfull=nfull, tail_sub=tail_sub)


# --------------------------------------------------------------- bass program

def _build_program(m_pad, nfull, tail_sub, layers, stride):
    # extra tail slack so window loads may harmlessly overread
    padw = CPAD + m_pad + CPAD + BLOCK + 128
    n_eff = nfull + (1 if tail_sub else 0)
    nc = bacc.Bacc("TRN2", target_bir_lowering=False, debug=False)

    x0_d = nc.dram_tensor("x0", (2, 128, padw), BF16, kind="ExternalInput")
    w0_d = nc.dram_tensor("w0p", (128, N_TAPS * 2 * HID), BF16,
                          kind="ExternalInput")
    wr_d = nc.dram_tensor("wrp", (max(layers - 1, 1), 128, N_TAPS * 4 * HID),
                          BF16, kind="ExternalInput")
    acg_d = nc.dram_tensor("acg", (layers, 32, 2048), F32R, kind="ExternalInput")
    bcgm_d = nc.dram_tensor("bcgm", (layers, 64, 2048), F32R,
                            kind="ExternalInput")
    smask_d = nc.dram_tensor("smask", (128, 256), F32R, kind="ExternalInput")
    msk64_d = nc.dram_tensor("msk64", (64, (nfull + 2) * SUB), F32R,
                             kind="ExternalInput")
    out_d = nc.dram_tensor("out", (4, 128, m_pad), DT.float32,
                           kind="ExternalOutput")
    xa_d = nc.dram_tensor("xa", (4, 128, padw), BF16, kind="Internal")
    xb_d = nc.dram_tensor("xb", (4, 128, padw), BF16, kind="Internal")

    deltas = [dy * stride + dx for dy, dx in OFFS]

    with tile.TileContext(nc) as tc:
        with (
            tc.tile_pool(name="consts", bufs=1) as constp,
            tc.tile_pool(name="wp", bufs=2) as wpool,
            tc.tile_pool(name="lyc", bufs=1) as lycp,
            tc.tile_pool(name="yb", bufs=3) as ypool,
            tc.tile_pool(name="ybf", bufs=2) as ybfpool,
            tc.tile_pool(name="tt", bufs=1) as ttpool,
            tc.tile_pool(name="tt2", bufs=2) as tt2pool,
            tc.tile_pool(name="psC", bufs=1, space=bass.MemorySpace.PSUM) as psCp,
            tc.tile_pool(name="psS", bufs=1, space=bass.MemorySpace.PSUM) as psSp,
            tc.tile_pool(name="psA", bufs=1, space=bass.MemorySpace.PSUM) as psAp,
            tc.tile_pool(name="psB", bufs=2, space=bass.MemorySpace.PSUM) as psBp,
        ):
            smask = constp.tile([128, 256], F32R)
            nc.sync.dma_start(smask[:], smask_d.ap())
            xw0 = constp.tile([128, 4 * WIN], BF16, tag="xw0")
            xw1 = constp.tile([128, 4 * WIN], BF16, tag="xw1")
            xwt = [xw0, xw1]
            # mask replicated in rows 0:32 and 32:64 per (parity, co);
            # ep_stats overwrites rows 0:32 with w32 so [w32; msk] pairs
            # with the [gm16; bc32] stack in one 64-contraction B matmul
            mwt = [[], []]
            for p in range(2):
                for co in range(4):
                    mw_pc = constp.tile([64, SUB], F32R, tag=f"mw{p}{co}",
                                        name=f"mw{p}{co}")
                    mwt[p].append(mw_pc)

            # zero the conv pads of the internal ping-pong buffers once
            zpad = constp.tile([128, CPAD], BF16)
            nc.gpsimd.memset(zpad[:], 0.0)
            for buf in (xa_d, xb_d):
                for ci in range(4):
                    nc.sync.dma_start(buf.ap()[ci, :, 0:CPAD], zpad[:])
                    for z0 in range(CPAD + m_pad, padw, CPAD):
                        zw = min(CPAD, padw - z0)
                        nc.sync.dma_start(buf.ap()[ci, :, z0:z0 + zw],
                                          zpad[:, 0:zw])

            state = {"cur": 0}

            def load_xw(p, src_aps, nci_, bexpr):
                for ci in range(nci_):
                    nc.sync.dma_start(
                        xwt[p][:, ci * WIN:(ci + 1) * WIN],
                        src_aps[ci][:, bass.ds(bexpr * BLOCK + (CPAD - 67),
                                               WIN)])

            def load_masks(p, bexpr):
                for co in range(4):
                    nc.sync.dma_start(
                        mwt[p][co][:],
                        msk64_d.ap()[:, bass.ds(bexpr * SUB, SUB)])

            def load_weights(li):
                nci_ = 2 if li == 0 else 4
                w_sb = wpool.tile([128, N_TAPS * 4 * HID], BF16, tag="w")
                wsrc = (w0_d.ap() if li == 0
                        else wr_d.ap()[li - 1, :, 0:N_TAPS * 4 * HID])
                wq = N_TAPS * nci_ * 128  # cols per co chunk
                for co in range(4):
                    nc.gpsimd.dma_start(w_sb[:, co * wq:(co + 1) * wq],
                                        wsrc[:, co * wq:(co + 1) * wq])
                return w_sb

            # GroupNorm epilogue queues, software-pipelined across block AND
            # layer boundaries so the PE never drains at either.
            ep_q = {"stat": [], "ab": []}

            def ep_push(stat_fn):
                ep_q["stat"].append(stat_fn)
                if len(ep_q["stat"]) > 1:
                    ep_q["ab"].append(ep_q["stat"].pop(0)())
                if len(ep_q["ab"]) > 1:
                    ep_q["ab"].pop(0)()

            def ep_drain():
                while ep_q["stat"]:
                    ep_q["ab"].append(ep_q["stat"].pop(0)())
                    if len(ep_q["ab"]) > 1:
                        ep_q["ab"].pop(0)()
                while ep_q["ab"]:
                    ep_q["ab"].pop(0)()

            def run_layer(li, nci, src_aps, dst_aps, final, w_sb, w_next):
                acg_sb = lycp.tile([32, 2048], F32R, tag="acg")
                nc.gpsimd.dma_start(acg_sb[:], acg_d.ap()[li])
                bcgm_sb = lycp.tile([64, 2048], F32R, tag="bcgm")
                nc.gpsimd.dma_start(bcgm_sb[:], bcgm_d.ap()[li])

                def conv_half(co, j0, j1, ps, cur):
                    nmm = nci * N_TAPS
                    mi = 0
                    for ci in range(nci):
                        for k in range(N_TAPS):
                            woff = (co * nci * N_TAPS + k * nci + ci) * 128
                            lhsT = w_sb[:, woff:woff + 128]
                            base = ci * WIN + 67 + deltas[k]
                            for j in range(j0, j1):
                                nc.tensor.matmul(
                                    ps[:, (j - j0) * SUB:(j - j0 + 1) * SUB],
                                    lhsT,
                                    xwt[cur][:, base + j * SUB:
                                              base + j * SUB + SUB],
                                    start=(mi == 0), stop=(mi == nmm - 1))
                            mi += 1

                def ep_stats(co, y2t, mw_t, nsub):
                    # psX2[:, 0:512] = sum y per group; [:, 512:1024] = E[y^2]
                    # (the h=1 smask slices carry the 1/16 scaling)
                    psX2 = psSp.tile([32, 2 * SUB], DT.float32, tag="sxx")
                    for h in range(2):
                        for j in range(nsub):
                            nc.tensor.matmul(psX2[:, h * SUB:(h + 1) * SUB],
                                             smask[:, h * 128 + j * 32:
                                                   h * 128 + (j + 1) * 32],
                                             y2t[:, h, j * SUB:(j + 1) * SUB],
                                             start=(j == 0),
                                             stop=(j == nsub - 1))
                    sxs = ttpool.tile([32, SUB], F32R, tag="sxs")
                    nc.vector.tensor_copy(sxs[:], psX2[:, 0:SUB])
                    a = ttpool.tile([32, SUB], DT.float32, tag="a")
                    nc.vector.scalar_tensor_tensor(
                        a[:], sxs[:], -1.0 / (GSIZE * GSIZE), sxs[:],
                        mybir.AluOpType.mult, mybir.AluOpType.mult)  # -mu^2
                    uu = ttpool.tile([32, SUB], DT.float32, tag="uu")
                    nc.vector.scalar_tensor_tensor(
                        uu[:], psX2[:, SUB:2 * SUB], EPS, a[:],
                        mybir.AluOpType.add, mybir.AluOpType.add)  # var+eps
                    r = ttpool.tile([32, SUB], DT.float32, tag="r")
                    nc.vector.reciprocal_approx_fast(r[:], uu[:])
                    inv = ttpool.tile([32, SUB], DT.float32, tag="a")
                    nc.scalar.activation(inv[:], r[:],
                                         mybir.ActivationFunctionType.Sqrt)
                    invm = tt2pool.tile([32, SUB], F32R, tag="invm")
                    nc.vector.tensor_tensor(invm[:], inv[:], mw_t[0:32, :],
                                            mybir.AluOpType.mult)
                    # w32 = (sum y)*inv*msk overwrites the rows-0:32 mask copy
                    nc.vector.tensor_tensor(mw_t[0:32, :], sxs[:], invm[:],
                                            mybir.AluOpType.mult)
                    return invm, mw_t

                def ep_ab(co, y2t, invm, mw_t, bexpr, nsub):
                    ybf = (None if final
                           else ybfpool.tile([128, BLOCK], BF16, tag="ybf"))
                    for j in range(nsub):
                        cj = co * 512 + j * 128
                        psA = psAp.tile([128, SUB], DT.float32, tag="A")
                        nc.tensor.matmul(psA[:], acg_sb[:, cj:cj + 128],
                                         invm[:], start=True, stop=True)
                        psB = psBp.tile([128, SUB], DT.float32, tag="B")
                        nc.tensor.matmul(psB[:], bcgm_sb[:, cj:cj + 128],
                                         mw_t[:], start=True, stop=True)
                        t1 = tt2pool.tile([128, SUB], DT.float32, tag="t1")
                        nc.vector.tensor_tensor(
                            t1[:], psA[:], y2t[:, 0, j * SUB:(j + 1) * SUB],
                            mybir.AluOpType.mult)
                        t2 = tt2pool.tile([128, SUB], DT.float32, tag="t2")
                        nc.vector.tensor_tensor(t2[:], psB[:], t1[:],
                                                mybir.AluOpType.add)
                        nc.scalar.activation(
                            (y2t[:, 0, j * SUB:(j + 1) * SUB] if final
                             else ybf[:, j * SUB:(j + 1) * SUB]),
                            t2[:], mybir.ActivationFunctionType.Relu)

                    dst = dst_aps[co][:, bass.ds(bexpr * BLOCK + (0 if final
                                                                  else CPAD),
                                                 nsub * SUB)]
                    # trigger output writes from the ACT engine's DGE queue so
                    # the sync queue carries only loads (keeps the window
                    # prefetch from queuing behind this block's writes)
                    nc.scalar.dma_start(
                        dst,
                        ybf[:, 0:nsub * SUB] if not final
                        else y2t[:, 0, 0:nsub * SUB].bitcast(DT.float32))

                def run_block(bexpr, nsub, pre):
                    cur = state["cur"]
                    if pre is not None:
                        if pre[0] == "same":
                            load_xw(1 - cur, src_aps, nci, pre[1])
                        else:  # next layer's block 0 (reads this layer's dst)
                            load_xw(1 - cur, pre[1], 4, 0)
                    for co in range(4):
                        y2t = ypool.tile([128, 2, BLOCK], F32R, tag="y")
                        haj = min(2, nsub)
                        psa = psCp.tile([128, HA], DT.float32, tag="ca")
                        conv_half(co, 0, haj, psa, cur)
                        nc.vector.tensor_copy(y2t[:, 0, 0:haj * SUB],
                                              psa[:, 0:haj * SUB])
                        # square directly from PSUM on ACT, in parallel with
                        # the DVE copy (not serialized behind it)
                        nc.scalar.square(y2t[:, 1, 0:haj * SUB],
                                         psa[:, 0:haj * SUB])
                        if nsub == 3:
                            psb = psCp.tile([128, SUB], DT.float32, tag="cb")
                            conv_half(co, 2, 3, psb, cur)
                            nc.vector.tensor_copy(y2t[:, 0, HA:BLOCK], psb[:])
                            nc.scalar.square(y2t[:, 1, HA:BLOCK], psb[:])

                        def mk_stat(co=co, y2t=y2t, mw_t=mwt[cur][co],
                                    bexpr=bexpr, nsub=nsub):
                            def do_stat():
                                st = ep_stats(co, y2t, mw_t, nsub)

                                def do_ab():
                                    ep_ab(co, y2t, *st, bexpr, nsub)
                                return do_ab
                            return do_stat

                        ep_push(mk_stat())
                    # mask prefetch emitted after the carried ep work of the
                    # previous block (same tile parity) to keep emission
                    # order consistent with the semantic order
                    if pre is not None:
                        load_masks(1 - cur, 0 if pre[0] == "next" else pre[1])
                    state["cur"] = 1 - cur

                # fully unrolled block loop: no For_i iteration-transition
                # overhead, and the GN epilogue pipeline carries across every
                # block boundary of the layer (drained once per layer).
                for b in range(n_eff):
                    nsub = 3 if b < nfull else tail_sub
                    if b < n_eff - 1:
                        pre = ("same", b + 1)
                    elif not final:
                        pre = ("next", dst_aps)
                    else:
                        pre = None
                    run_block(b, nsub, pre)
                ep_drain()

            bufs = {"x0": x0_d, "xa": xa_d, "xb": xb_d}
            seq = ["x0"] + ["xa", "xb"] * 4
            w_cur = load_weights(0)
            load_xw(0, [x0_d.ap()[ci] for ci in range(2)], 2, 0)
            load_masks(0, 0)
            for li in range(layers):
                src, dst = seq[li], seq[li + 1]
                nci = 2 if li == 0 else 4
                src_aps = [bufs[src].ap()[ci] for ci in range(nci)]
                final = li == layers - 1
                dst_aps = ([out_d.ap()[co] for co in range(4)] if final
                           else [bufs[dst].ap()[co] for co in range(4)])
                w_next = None if final else load_weights(li + 1)
                run_layer(li, nci, src_aps, dst_aps, final, w_cur, w_next)
                w_cur = w_next
            ep_drain()

    nc.compile()
    return nc


# ------------------------------------------------------------- host packing

def _pack_host(inputs, pos, meta, layers=8):
    bf16 = mybir.dt.np(BF16)
    feats = np.ascontiguousarray(np.asarray(inputs["features"], np.float32))
    w0 = np.asarray(inputs["w0"], np.float32)
    w_rest = np.asarray(inputs["w_rest"], np.float32)
    gamma = np.asarray(inputs["gamma"], np.float32)
    beta = np.asarray(inputs["beta"], np.float32)
    n, cin = feats.shape
    stride, r8, m_pad = meta["stride"], meta["r8"], meta["m_pad"]
    nfull = meta["nfull"]
    rgst = meta["rg"] * stride
    padw = CPAD + m_pad + CPAD + BLOCK + 128

    x_g = np.zeros((cin, rgst), np.float32)
    x_g[:, pos] = feats.T
    mask_g = np.zeros(rgst, np.float32)
    mask_g[pos] = 1.0

    # weights: wpk[p, co, k, ci, :] = w[k, ci*128+p, co*128:(co+1)*128]
    w0p = np.ascontiguousarray(
        w0.reshape(N_TAPS, 2, 128, 4, 128).transpose(2, 3, 0, 1, 4)
    ).reshape(128, N_TAPS * 2 * HID).astype(bf16)
    nl = max(layers - 1, 1)
    wrp = np.ascontiguousarray(
        w_rest[:layers - 1].reshape(layers - 1, N_TAPS, 4, 128, 4, 128)
        .transpose(0, 3, 4, 1, 2, 5)
    ).reshape(layers - 1, 128, N_TAPS * 4 * HID).astype(bf16)
    if wrp.shape[0] < nl:
        wrp = np.zeros((nl, 128, N_TAPS * 4 * HID), bf16)

    ch = np.arange(128)
    acg = np.zeros((layers, 32, 4, 4, 128), np.float32)
    gm16 = np.zeros((layers, 32, 4, 4, 128), np.float32)
    bc32 = np.zeros((layers, 32, 4, 4, 128), np.float32)
    for li in range(layers):
        for co in range(4):
            g_ = gamma[li, co * 128:(co + 1) * 128]
            b_ = beta[li, co * 128:(co + 1) * 128]
            for j in range(4):
                rows = 8 * j + ch // GSIZE
                acg[li, rows, co, j, ch] = g_
                gm16[li, rows, co, j, ch] = -g_ / GSIZE
                bc32[li, 8 * j, co, j, :] = b_
    acg = acg.reshape(layers, 32, 2048)
    # B matmul stack: rows 0:32 pair with w32, rows 32:64 with the mask
    bcgm = np.concatenate([gm16.reshape(layers, 32, 2048),
                           bc32.reshape(layers, 32, 2048)], axis=1)

    # stats mask: out partition 8j + g sums group g of subtile j.
    # cols 0:128 (h=0, sum y) carry 1.0; cols 128:256 (h=1) carry 1/16
    # so the second stats chain lands as E[y^2] directly.
    smask = np.zeros((2, 128, 4, 32), np.float32)
    for j in range(4):
        smask[0, ch, j, 8 * j + ch // GSIZE] = 1.0
        smask[1, ch, j, 8 * j + ch // GSIZE] = 1.0 / GSIZE
    smask = smask.transpose(1, 0, 2, 3).reshape(128, 256)

    in_maps = []
    mpb = (nfull + 1) * BLOCK
    for s in range(N_CORES):
        c0 = s * r8 * stride
        x0 = np.zeros((2, 128, padw), bf16)
        seg = x_g[:, c0:min(c0 + m_pad, rgst)]
        x0[:, :, CPAD:CPAD + seg.shape[1]] = seg.reshape(2, 128, -1).astype(bf16)
        mc = np.zeros(mpb, np.float32)
        mseg = mask_g[c0:min(c0 + mpb, rgst)]
        mc[:mseg.shape[0]] = mseg
        # msk64[8j+g, b*512+c] = msk64[32+8j+g, ...] = mask[b*1536 + j*512 + c]
        # (rows 0:32 get overwritten by w32 on-chip; rows 32:64 stay mask)
        m4 = mc.reshape(-1, NSUB, SUB)  # [nb, j, c]
        msk64 = np.zeros((64, (nfull + 2) * SUB), np.float32)
        for j in range(NSUB):
            for g in range(8):
                msk64[8 * j + g, :mpb // NSUB] = m4[:, j, :].reshape(-1)
                msk64[32 + 8 * j + g, :mpb // NSUB] = m4[:, j, :].reshape(-1)
        in_maps.append({
            "x0": x0, "w0p": w0p, "wrp": wrp, "acg": acg, "bcgm": bcgm,
            "smask": smask, "msk64": msk64,
        })
    return in_maps


TRACE = False
LAST_RESULT = {}


def kernel(**inputs) -> np.ndarray:
    nbr = np.asarray(inputs["nbr_idx"])
    n = nbr.shape[0]
    pos, meta = _build_canvas_map(nbr)
    in_maps = _pack_host(inputs, pos, meta)
    nc = _build_program(meta["m_pad"], meta["nfull"], meta["tail_sub"], 8,
                        meta["stride"])
    res = run_bass_kernel_spmd(nc, in_maps, list(range(N_CORES)), trace=TRACE)
    LAST_RESULT["exec_time_ns"] = res.exec_time_ns
    LAST_RESULT["profile_json"] = res.profile_json

    stride, r8 = meta["stride"], meta["r8"]
    row = pos // stride
    own = np.clip((row - HALO_ROWS) // r8, 0, N_CORES - 1)
    result = np.zeros((n, HID), np.float32)
    for s in range(N_CORES):
        sel = own == s
        local = pos[sel] - s * r8 * stride
        o = res.results[s]["out"]  # [4, 128, m_pad]
        result[sel] = o[:, :, local].reshape(HID, -1).T
    return result


if __name__ == "__main__":
    import reference

    inputs = reference.setup_inputs()
    out = kernel(**{k: np.asarray(v) for k, v in inputs.items()})
    exp = np.asarray(reference.reference(**inputs))
    err = np.linalg.norm(out - exp) / np.linalg.norm(exp)
    print(f"l2 rel err: {err:.3e}")
